# revision 24
# baseline (speedup 1.0000x reference)
"""Trainium2 Bass kernel for hypergraph message passing (gnn_message_passing).

Computes, for feature [N,E], adj [N,H], w1..w3 [H,H] (N=200000, E=H=128):
    f1 = leaky(adj.T @ feature)
    f2 = leaky(w1 @ f1) + f1
    f3 = leaky(w2 @ f2) + f2
    f4 = leaky(w3 @ f3) + f3
    out = leaky(adj @ f4)
with leaky(x) = max(x, 0.05*x).

Distribution: shard N across 8 NeuronCores (data parallel). adj.T@feature is
computed per-shard and AllReduce-summed ([H,E] = 64KB); the [H,H] stages are
replicated; adj@f4 is local per shard.

Schedule per core:
- A tiny dummy AllReduce is issued first: the first collective in a NEFF pays
  a ~75us ncfw warmup; warming it up under the phase-1 loads makes the real
  AllReduce cost ~10us.
- Phase 1 streams feature+adj via gpsimd casting-DMA (fp32 HBM -> bf16 SBUF,
  line rate) and accumulates adj.T@feature in PSUM with bf16 matmuls. adj is
  kept resident in SBUF (bf16), and each 128-row chunk is PE-transposed into
  a resident adjT buffer (PSUM->SBUF copies alternate ScalarE/VectorE).
- Real AllReduce + the three small stages (fp32).
- Phase 3: per 7-chunk batch, PE matmuls adjT_chunk.T @ f4 into PSUM, ScalarE
  emits 0.05*z, VectorE takes max(z, 0.05*z) = leaky, stores stream out.
"""

import sys

if "/opt/trn_rl_repo" not in sys.path:
    sys.path.insert(0, "/opt/trn_rl_repo")

import numpy as np

import concourse.bass as bass
import concourse.mybir as mybir
import concourse.tile as tile
from concourse import bacc
from concourse.bass import ts
from concourse.bass_utils import run_bass_kernel_spmd
from concourse.masks import make_identity

N, E, H = 200000, 128, 128
N_CORES = 8
N_PC = N // N_CORES            # 25000 rows per core
CHUNK = 128
N_CHUNKS = -(-N_PC // CHUNK)   # 196
N_LOC = N_CHUNKS * CHUNK       # 25088 (pad 88 zero rows)
GROUP = 14                     # chunks per DMA group (~0.9MB fp32 reads)
N_GROUPS = N_CHUNKS // GROUP   # 14
NEG = 0.05

F32 = mybir.dt.float32
BF16 = mybir.dt.bfloat16

_CACHE = {}
LAST_RESULTS = None


def _build():
    nc = bacc.Bacc(
        "TRN2", target_bir_lowering=False, debug=False, num_devices=N_CORES
    )
    feature = nc.dram_tensor("feature", [N_LOC, E], F32, kind="ExternalInput")
    adj = nc.dram_tensor("adj", [N_LOC, H], F32, kind="ExternalInput")
    w_in = [
        nc.dram_tensor(f"w{i}", [H, H], F32, kind="ExternalInput")
        for i in (1, 2, 3)
    ]
    out = nc.dram_tensor("out", [N_LOC, E], F32, kind="ExternalOutput")

    # DRAM views: partition p takes GROUP consecutive rows, chunk n is the
    # row-within-p. The N-contraction and the per-row phase 3 are invariant
    # to which rows land in which chunk, and this gives the DMA one
    # contiguous 7KB run per partition instead of 14 512B runs.
    feat_v = feature.ap().rearrange("(g p n) e -> g p n e", p=CHUNK, n=GROUP)
    adj_v = adj.ap().rearrange("(g p n) e -> g p n e", p=CHUNK, n=GROUP)
    out_v = out.ap().rearrange("(g p n) e -> g p n e", p=CHUNK, n=GROUP)

    RG = [list(range(N_CORES))]

    with tile.TileContext(nc) as tc:
        with (
            tc.tile_pool(name="const", bufs=1) as cpool,
            tc.tile_pool(name="adjs", bufs=1) as apool,
            tc.tile_pool(name="loads", bufs=3) as lpool,
            tc.tile_pool(name="outs", bufs=4) as opool,
            tc.tile_pool(name="ps", bufs=3, space="PSUM") as pspool,
            tc.tile_pool(name="ops", bufs=2, space="PSUM") as opspool,
            tc.tile_pool(name="f1p", bufs=1, space="PSUM") as f1pool,
            tc.tile_pool(name="dram", bufs=1, space="DRAM") as dpool,
        ):
            # ---- dummy collective: pays the one-time ncfw warmup (~75us)
            # under the phase-1 loads. Its input is anchored on group 0's
            # load tile so it fires ~t=20us and completes right as f1 is
            # ready — collective warmth decays, so the real AllReduce must
            # follow the dummy as closely as possible.
            dmy = cpool.tile([128, 16], F32, tag="dmy")
            dmy_in = dpool.tile([128, 16], F32, tag="dmyin")
            dmy_out = dpool.tile([128, 16], F32, tag="dmyout")

            ident_f = cpool.tile([128, 128], F32, tag="identf")
            make_identity(nc, ident_f[:])
            ident_b = cpool.tile([128, 128], BF16, tag="identb")
            nc.vector.tensor_copy(out=ident_b[:], in_=ident_f[:])

            # ---- weights: load + PE transpose (w @ x needs wT as lhsT) ----
            wT = []
            for i in range(3):
                wsb = cpool.tile([128, 128], F32, tag=f"w{i}")
                nc.sync.dma_start(out=wsb[:], in_=w_in[i].ap())
                wps = pspool.tile([128, 128], F32, tag="ps")
                nc.tensor.transpose(wps[:], wsb[:], ident_f[:])
                wt = cpool.tile([128, 128], F32, tag=f"wt{i}")
                nc.vector.tensor_copy(out=wt[:], in_=wps[:])
                wT.append(wt)

            # ---- phase 1: stream loads (cast to bf16), accumulate f1,
            #      transpose every adj chunk into resident adjT ----
            adj_all = apool.tile([128, N_LOC], BF16, tag="adj_all")
            adjT = apool.tile([128, N_LOC], BF16, tag="adjT")
            f1ps = f1pool.tile([128, 128], F32, tag="f1ps")
            for g in range(N_GROUPS):
                # feature: HWDGE fp32 load (RTL descriptor gen) + DVE cast,
                # adj: SWDGE casting load — two generators run in parallel
                # so the combined read stream reaches the HBM limit.
                ft32 = lpool.tile([128, GROUP * CHUNK], F32, tag="ft32")
                nc.sync.dma_start(
                    out=ft32[:].rearrange("p (n e) -> p n e", n=GROUP),
                    in_=feat_v[g],
                )
                ft = lpool.tile([128, GROUP * CHUNK], BF16, tag="ft")
                nc.vector.tensor_copy(out=ft[:], in_=ft32[:])
                ag = adj_all[:, ts(g, GROUP * CHUNK)]
                nc.gpsimd.dma_start(
                    out=ag.rearrange("p (n e) -> p n e", n=GROUP),
                    in_=adj_v[g],
                )
                if g == 2:
                    # launch the warmup collective ~t=25us so it completes
                    # right as f1 is ready (~t=100)
                    nc.scalar.copy(out=dmy[:], in_=ft32[:, :16])
                    nc.sync.dma_start(out=dmy_in[:], in_=dmy[:])
                    nc.gpsimd.collective_compute(
                        "AllReduce",
                        mybir.AluOpType.add,
                        replica_groups=RG,
                        ins=[dmy_in.opt()],
                        outs=[dmy_out.opt()],
                    )
                for n in range(GROUP):
                    c = g * GROUP + n
                    ach = adj_all[:, ts(c, CHUNK)]
                    nc.tensor.matmul(
                        f1ps[:],
                        lhsT=ach,
                        rhs=ft[:, ts(n, CHUNK)],
                        start=(c == 0),
                        stop=(c == N_CHUNKS - 1),
                        skip_group_check=True,
                    )
                # transposes: batch 7 chunks into one shared PSUM tile so the
                # PSUM->SBUF copy is one wide op; alternate ACT/DVE per batch
                for b in range(GROUP // 7):
                    tps = opspool.tile([128, 7 * CHUNK], BF16, tag="ops")
                    for k in range(7):
                        c = g * GROUP + b * 7 + k
                        nc.tensor.transpose(
                            tps[:, ts(k, CHUNK)],
                            adj_all[:, ts(c, CHUNK)],
                            ident_b[:],
                        )
                    dst = adjT[:, bass.ds((g * GROUP + b * 7) * CHUNK, 7 * CHUNK)]
                    if b % 2 == 0:
                        nc.scalar.copy(out=dst, in_=tps[:])
                    else:
                        nc.vector.tensor_copy(out=dst, in_=tps[:])

            # ---- real AllReduce of the [H,E] partial over the 8 cores ----
            f1sb = cpool.tile([128, 128], F32, tag="f1sb")
            nc.scalar.copy(out=f1sb[:], in_=f1ps[:])
            cc_in = dpool.tile([128, 128], F32, tag="ccin")
            cc_out = dpool.tile([128, 128], F32, tag="ccout")
            nc.sync.dma_start(out=cc_in[:], in_=f1sb[:])
            nc.gpsimd.collective_compute(
                "AllReduce",
                mybir.AluOpType.add,
                replica_groups=RG,
                ins=[cc_in.opt()],
                outs=[cc_out.opt()],
            )
            f1r = cpool.tile([128, 128], F32, tag="f1r")
            nc.sync.dma_start(out=f1r[:], in_=cc_out[:])

            # leaky(x) = max(0.05x, x)
            f1 = cpool.tile([128, 128], F32, tag="f1")
            nc.vector.scalar_tensor_tensor(
                out=f1[:], in0=f1r[:], scalar=NEG, in1=f1r[:],
                op0=mybir.AluOpType.mult, op1=mybir.AluOpType.max,
            )

            # ---- phase 2: three replicated [H,H] hyperweight stages ----
            fprev = f1
            for i in range(3):
                sps = pspool.tile([128, 128], F32, tag="ps")
                nc.tensor.matmul(
                    sps[:], lhsT=wT[i][:], rhs=fprev[:],
                    start=True, stop=True, skip_group_check=True,
                )
                t1 = cpool.tile([128, 128], F32, tag=f"s{i}a")
                nc.scalar.copy(out=t1[:], in_=sps[:])
                t2 = cpool.tile([128, 128], F32, tag=f"s{i}b")
                nc.vector.scalar_tensor_tensor(
                    out=t2[:], in0=t1[:], scalar=NEG, in1=t1[:],
                    op0=mybir.AluOpType.mult, op1=mybir.AluOpType.max,
                )
                fnext = cpool.tile([128, 128], F32, tag=f"f{i + 2}")
                nc.vector.tensor_add(out=fnext[:], in0=t2[:], in1=fprev[:])
                fprev = fnext

            # f4 in bf16 for the phase-3 matmuls
            f4b = cpool.tile([128, 128], BF16, tag="f4b")
            nc.vector.tensor_copy(out=f4b[:], in_=fprev[:])

            # ---- phase 3: out = leaky(adj @ f4), 7-chunk batches ----
            BATCH = 7
            for g in range(N_GROUPS):
                osb = opool.tile([128, GROUP * CHUNK], F32, tag="osb")
                osb_v = osb[:].rearrange("p (n e) -> p n e", n=GROUP)
                for b in range(GROUP // BATCH):
                    ops = opspool.tile([128, BATCH * CHUNK], F32, tag="ops")
                    for k in range(BATCH):
                        c = g * GROUP + b * BATCH + k
                        nc.tensor.matmul(
                            ops[:, ts(k, CHUNK)],
                            lhsT=adjT[:, ts(c, CHUNK)],
                            rhs=f4b[:],
                            start=True,
                            stop=True,
                            skip_group_check=True,
                        )
                    tb = opool.tile([128, BATCH * CHUNK], F32, tag="tb")
                    nc.scalar.activation(
                        out=tb[:], in_=ops[:],
                        func=mybir.ActivationFunctionType.Copy, scale=NEG,
                    )
                    nc.vector.tensor_max(
                        out=osb[:, ts(b, BATCH * CHUNK)], in0=ops[:], in1=tb[:]
                    )
                    # store each half as soon as its leaky is done;
                    # alternate the two HWDGE rings (SP / ACT)
                    dma_eng = nc.sync if b % 2 == 0 else nc.scalar
                    dma_eng.dma_start(
                        out=out_v[g][:, b * BATCH : (b + 1) * BATCH, :],
                        in_=osb_v[:, b * BATCH : (b + 1) * BATCH, :],
                    )

    nc.compile()
    return nc


def _get_nc():
    if "nc" not in _CACHE:
        _CACHE["nc"] = _build()
    return _CACHE["nc"]


def kernel(**inputs) -> np.ndarray:
    global LAST_RESULTS
    feature = np.asarray(inputs["feature"], dtype=np.float32)
    adj = np.asarray(inputs["adj"], dtype=np.float32)
    ws = {k: np.ascontiguousarray(np.asarray(inputs[k], dtype=np.float32))
          for k in ("w1", "w2", "w3")}

    nc = _get_nc()

    pad = N_LOC - N_PC
    in_maps = []
    for i in range(N_CORES):
        fs = feature[i * N_PC : (i + 1) * N_PC]
        as_ = adj[i * N_PC : (i + 1) * N_PC]
        if pad:
            z = np.zeros((pad, E), np.float32)
            fs = np.concatenate([fs, z], axis=0)
            as_ = np.concatenate([as_, z], axis=0)
        in_maps.append(
            {
                "feature": np.ascontiguousarray(fs),
                "adj": np.ascontiguousarray(as_),
                **ws,
            }
        )

    res = run_bass_kernel_spmd(nc, in_maps, core_ids=list(range(N_CORES)))
    LAST_RESULTS = res
    parts = [res.results[i]["out"][:N_PC] for i in range(N_CORES)]
    return np.concatenate(parts, axis=0)
